# revision 1
# baseline (speedup 1.0000x reference)
"""Two-layer GAT (GATConv 128->64x4 concat, relu, GATConv 256->2) on 8 TRN2
NeuronCores, self-contained.

Sharding: edges partitioned by destination node; core c owns dst nodes
[c*6250, (c+1)*6250). Within a core, own nodes are sorted by in-degree and
grouped into 49 windows of 128 ("slots"). Edges of a node are laid out in
"rounds": round r, slot s holds the r-th in-edge of window-node s, so each
[128-edge] round tile is already dst-aligned — segment aggregation is a plain
PSUM accumulation with an identity stationary operand (no one-hot matmuls),
and al_dst comes from one 128-row gather per window instead of a per-edge
gather. Degree sorting makes rounds-per-window track the window's max degree,
so padding stays low. Padding edges point at a dedicated table row whose
al_src is -1e30, which drives exp(leakyrelu(e)) to zero.

Device pipeline per core:
  Phase A: tab[g] = [x@W1 as bf16 (512B) | al_src f32[4] | al_dst f32[4] | pad]
           (768B rows) for ALL nodes, from a host-pretransposed bf16 x.
  Phase B: per window: gather src rows, e = al_s[src] + al_d[dst](broadcast),
           p = exp(leaky_relu(e)), msg = h[src]*p (bf16), PSUM += msg via
           identity matmul; readout: alpha-normalize, +b1, relu,
           h2lite = relu1 @ [W2|W2 a_s2|W2 a_d2] -> h2own rows.
  AllGather h2own -> h2full. Phase C: same structure for layer 2.
"""

import os
import sys
import time

sys.path.insert(0, "/opt/trn_rl_repo")

import numpy as np
import ml_dtypes

import concourse.bacc as bacc
import concourse.mybir as mybir
import concourse.tile as tile
from concourse.masks import make_identity

# problem constants (hardcoded per harness contract)
N = 50000
INCH = 128
HID = 64
HEADS = 4
OUT = 2
NEG = 0.2
CORES = 8
NPC = N // CORES          # 6250 dst nodes per core
P = 128
SL = 127                  # real slots per window; slot 127 is always padding so
                          # every gather chunk's final int16 index is >= 0
W = 50                    # windows per core (50*127 = 6350 >= 6250)
NPCP = W * P              # padded nodes per core (6400)
NPCP2 = NPCP + P          # + one pad-row block (6528)
NROW1 = 393 * P           # l1 table rows (50304 > N)
NROW2 = CORES * NPCP2     # h2 table rows (52224)
PADROW1 = N               # tab row used by padding edges (al_s = -1e30); >= 32768
PAD2 = 5 * NPCP2 + NPCP   # h2full pad row inside core 5's block; >= 32768
BIAS = 32768              # int16 gather index bias
EPS = 1e-16
ROWF = 192                # tab row width in f32 (768 B)

f32 = mybir.dt.float32
bf16 = mybir.dt.bfloat16
i16 = mybir.dt.int16

LAST_EXEC_NS = None
_cache = {}


def _wrap_idx_stream(a):
    """[C, M] int64 -> biased int16 [C, 128, M//16] (16-partition wrap,
    replicated to all 8 Q7 groups)."""
    Cn, M = a.shape
    b = (a - BIAS).astype(np.int16).reshape(Cn, M // 16, 16).transpose(0, 2, 1)
    return np.tile(b, (1, 8, 1)).copy()


CHUNK = int(os.environ.get("KCHUNK", "8"))


def _chunks(K):
    """[(tile_off, ntiles)] with ntiles <= CHUNK (dma_gather idx limit)."""
    out = []
    off = 0
    while off < K:
        n = min(CHUNK, K - off)
        out.append((off, n))
        off += n
    return out


def _build(R):
    R = list(R)
    # per window: 1 own-row round (al_dst / h2own values) + R edge rounds
    off = np.zeros(W + 1, np.int64)
    off[1:] = np.cumsum(np.asarray(R) + 1)
    S = int(off[-1])
    phases = os.environ.get("KPHASES", "ABGC")
    nqueues = int(os.environ.get("KQUEUES", "2"))
    nc = bacc.Bacc(
        "TRN2", target_bir_lowering=False, debug=False, num_devices=CORES,
        num_swdge_queues=nqueues,
    )

    # inputs
    xT_d = nc.dram_tensor("xT", [INCH, NROW1], bf16, kind="ExternalInput")
    wcat_d = nc.dram_tensor("wcat", [INCH, 264], bf16, kind="ExternalInput")
    w2cat_d = nc.dram_tensor("w2cat", [P, 8], f32, kind="ExternalInput")
    b1_d = nc.dram_tensor("b1", [1, 256], f32, kind="ExternalInput")
    b2_d = nc.dram_tensor("b2", [1, 2], f32, kind="ExternalInput")
    idx1_d = nc.dram_tensor("idx1", [P, S * 8], i16, kind="ExternalInput")
    idx2_d = nc.dram_tensor("idx2", [P, S * 8], i16, kind="ExternalInput")

    out_d = nc.dram_tensor("out", [NPCP, OUT], f32, kind="ExternalOutput")

    # scratch
    tab = nc.dram_tensor("tab", [NROW1, ROWF], f32)
    h2own = nc.dram_tensor("h2own", [NPCP2, 64], f32)
    h2full = nc.dram_tensor("h2full", [NROW2, 64], f32, addr_space="Shared")

    LR = mybir.AluOpType
    AF = mybir.ActivationFunctionType

    with tile.TileContext(nc) as tc:
        with tc.tile_pool(name="const", bufs=1) as cpool:
            ident_b = cpool.tile([P, P], bf16)
            make_identity(nc, ident_b[:])
            ident_f = cpool.tile([P, P], f32)
            make_identity(nc, ident_f[:])
            ones = cpool.tile([1, P], f32)
            nc.vector.memset(ones[:], 1.0)

            wcat_sb = cpool.tile([INCH, 264], bf16)
            nc.sync.dma_start(out=wcat_sb[:], in_=wcat_d[:, :])
            w2cat_sb = cpool.tile([P, 8], f32)
            nc.sync.dma_start(out=w2cat_sb[:], in_=w2cat_d[:, :])
            b1row = cpool.tile([1, 256], f32)
            nc.sync.dma_start(out=b1row[:], in_=b1_d[:, :])
            b2row = cpool.tile([1, 2], f32)
            nc.sync.dma_start(out=b2row[:], in_=b2_d[:, :])
            idx1_sb = cpool.tile([P, S * 8], i16)
            nc.sync.dma_start(out=idx1_sb[:], in_=idx1_d[:, :])
            idx2_sb = cpool.tile([P, S * 8], i16)
            nc.sync.dma_start(out=idx2_sb[:], in_=idx2_d[:, :])

            # pad-row payload: al_s = al_d = -1e30, h = 0
            padrow = cpool.tile([1, ROWF], f32)
            nc.vector.memset(padrow[:], 0.0)
            nc.vector.memset(padrow[:, 128:136], -1e30)
            pad2 = cpool.tile([P, 4], f32)
            nc.vector.memset(pad2[:], 0.0)
            nc.vector.memset(pad2[:, 2:4], -1e30)

            # replicated biases
            with tc.tile_pool(name="psum_b", bufs=1, space="PSUM") as psb:
                b1_ps = psb.tile([P, 256], f32, space="PSUM")
                nc.tensor.matmul(out=b1_ps[:], lhsT=ones[:], rhs=b1row[:], start=True, stop=True)
                b1_rep = cpool.tile([P, 256], f32)
                nc.vector.tensor_copy(b1_rep[:], b1_ps[:])
                b2_ps = psb.tile([P, 2], f32, space="PSUM")
                nc.tensor.matmul(out=b2_ps[:], lhsT=ones[:], rhs=b2row[:], start=True, stop=True)
                b2_rep = cpool.tile([P, 2], f32)
                nc.vector.tensor_copy(b2_rep[:], b2_ps[:])

            reps = int(os.environ.get("KREPS", "1"))
            for _rep in range(reps):
              # ---------------- Phase A: node features table ----------------
              if "A" in phases:
                  with (
                      tc.tile_pool(name="sbufA", bufs=4) as pa,
                      tc.tile_pool(name="psumA", bufs=4, space="PSUM") as ppa,
                  ):
                      ntile = NROW1 // P
                      for i0 in range(0, ntile, 2):
                          npair = min(2, ntile - i0)
                          xt = pa.tile([INCH, 2 * P], bf16, tag="xt")
                          nc.sync.dma_start(
                              out=xt[:, 0 : npair * P],
                              in_=xT_d[:, i0 * P : (i0 + npair) * P],
                          )
                          for j in range(npair):
                              i = i0 + j
                              h_ps = ppa.tile([P, 264], f32, space="PSUM")
                              nc.tensor.matmul(
                                  out=h_ps[:], lhsT=xt[:, j * P : (j + 1) * P],
                                  rhs=wcat_sb[:], start=True, stop=True,
                              )
                              stg = pa.tile([P, ROWF], f32, tag="stg")
                              nc.vector.tensor_copy(stg[:, 0:128].bitcast(bf16), h_ps[:, 0:256])
                              nc.vector.tensor_copy(stg[:, 128:136], h_ps[:, 256:264])
                              # cols 136:192 are padding: written as-is, never read back
                              nc.sync.dma_start(out=tab[i * P : (i + 1) * P, :], in_=stg[:])
                      # overwrite the pad row (al_s/al_d = -1e30)
                      nc.sync.dma_start(out=tab[PADROW1 : PADROW1 + 1, :], in_=padrow[:])

              # ---------------- Phase B: layer-1 edge aggregation ----------------
              if "B" in phases:
                  with (
                      tc.tile_pool(name="sbufB", bufs=3) as pb,
                      tc.tile_pool(name="sbufBp", bufs=2) as pbp,
                      tc.tile_pool(name="sbufBs", bufs=4) as pbs,
                      tc.tile_pool(name="psumAgg", bufs=3, space="PSUM") as pagg,
                      tc.tile_pool(name="psumT", bufs=2, space="PSUM") as pt,
                      tc.tile_pool(name="psumH", bufs=2, space="PSUM") as ph,
                  ):
                      # pad block of h2own (rows NPCP..NPCP2)
                      nc.sync.dma_start(out=h2own[NPCP:NPCP2, 0:4], in_=pad2[:])
                      for w in range(W):
                          K = R[w]
                          o8 = int(off[w]) * 8
                          gbuf = pb.tile([P, K + 1, ROWF], f32, tag="gbuf")
                          for (toff, ntl) in _chunks(K + 1):
                              nc.gpsimd.dma_gather(
                                  gbuf[:, toff : toff + ntl, :],
                                  tab[BIAS:, :],
                                  idx1_sb[:, o8 + toff * 8 : o8 + (toff + ntl) * 8],
                                  ntl * P,
                                  ntl * P,
                                  ROWF,
                                  queue_num=w % nqueues,
                              )
                          # e = al_s[src] + al_d[dst] (round 0 = own rows), lrelu, exp
                          alv = gbuf[:, 1:, 128:132]
                          nc.vector.tensor_tensor(
                              out=alv, in0=alv,
                              in1=gbuf[:, 0, 132:136].unsqueeze(1).broadcast_to([P, K, 4]),
                              op=LR.add,
                          )
                          nc.vector.scalar_tensor_tensor(
                              out=alv, in0=alv, scalar=NEG, in1=alv,
                              op0=LR.mult, op1=LR.max,
                          )
                          p32 = pbs.tile([P, K, 4], f32, tag="p32")
                          nc.scalar.activation(p32[:], alv, AF.Exp)
                          pcat = pbp.tile([P, K, 260], bf16, tag="pcat")
                          nc.vector.tensor_copy(pcat[:, :, 256:260], p32[:])
                          hview = gbuf[:, 1:, 0:128].bitcast(bf16)
                          for k in range(K):
                              for h in range(HEADS):
                                  if h < 3:
                                      nc.vector.tensor_scalar(
                                          out=pcat[:, k, h * HID : (h + 1) * HID],
                                          in0=hview[:, k, h * HID : (h + 1) * HID],
                                          scalar1=p32[:, k, h : h + 1],
                                          scalar2=None,
                                          op0=LR.mult,
                                      )
                                  else:
                                      nc.scalar.mul(
                                          pcat[:, k, h * HID : (h + 1) * HID],
                                          hview[:, k, h * HID : (h + 1) * HID],
                                          p32[:, k, h : h + 1],
                                      )
                          agg_ps = pagg.tile([P, 260], f32, space="PSUM", tag="agg")
                          for k in range(K):
                              nc.tensor.matmul(
                                  out=agg_ps[:], lhsT=ident_b[:], rhs=pcat[:, k, :],
                                  start=(k == 0), stop=(k == K - 1),
                              )
                          # window readout
                          den = pbs.tile([P, 4], f32, tag="den")
                          nc.vector.tensor_scalar(
                              out=den[:], in0=agg_ps[:, 256:260], scalar1=EPS,
                              scalar2=None, op0=LR.add,
                          )
                          rec = pbs.tile([P, 4], f32, tag="rec")
                          nc.vector.reciprocal(rec[:], den[:])
                          relu1 = pbs.tile([P, 256], f32, tag="relu1")
                          for h in range(HEADS):
                              if h < 2:
                                  nc.vector.tensor_scalar(
                                      out=relu1[:, h * HID : (h + 1) * HID],
                                      in0=agg_ps[:, h * HID : (h + 1) * HID],
                                      scalar1=rec[:, h : h + 1],
                                      scalar2=None,
                                      op0=LR.mult,
                                  )
                              else:
                                  nc.scalar.mul(
                                      relu1[:, h * HID : (h + 1) * HID],
                                      agg_ps[:, h * HID : (h + 1) * HID],
                                      rec[:, h : h + 1],
                                  )
                          nc.vector.tensor_tensor(
                              out=relu1[:], in0=relu1[:], in1=b1_rep[:], op=LR.add
                          )
                          nc.scalar.activation(relu1[:], relu1[:], AF.Relu)
                          h2_ps = ph.tile([P, 4], f32, space="PSUM", tag="h2")
                          for half in range(2):
                              rT_ps = pt.tile([P, P], f32, space="PSUM", tag="ohT")
                              nc.tensor.transpose(
                                  out=rT_ps[:], in_=relu1[:, half * P : (half + 1) * P],
                                  identity=ident_f[:],
                              )
                              rT = pbs.tile([P, P], f32, tag="ohTs")
                              nc.vector.tensor_copy(rT[:], rT_ps[:])
                              nc.tensor.matmul(
                                  out=h2_ps[:], lhsT=rT[:],
                                  rhs=w2cat_sb[:, half * 4 : (half + 1) * 4],
                                  start=(half == 0), stop=(half == 1),
                              )
                          h2st = pbs.tile([P, 4], f32, tag="h2st")
                          nc.vector.tensor_copy(h2st[:], h2_ps[:])
                          nc.sync.dma_start(
                              out=h2own[w * P : (w + 1) * P, 0:4], in_=h2st[:]
                          )

              # ---------------- AllGather h2lite ----------------
              if "G" in phases:
                  nc.gpsimd.collective_compute(
                      "AllGather",
                      mybir.AluOpType.bypass,
                      replica_groups=[list(range(CORES))],
                      ins=[h2own.ap().opt()],
                      outs=[h2full.ap().opt()],
                  )

              # ---------------- Phase C: layer-2 edge aggregation ----------------
              if "C" in phases:
                  with (
                      tc.tile_pool(name="sbufC", bufs=3) as pc,
                      tc.tile_pool(name="sbufCs", bufs=4) as pcs,
                  ):
                      for w in range(W):
                          K = R[w]
                          o8 = int(off[w]) * 8
                          g2 = pc.tile([P, K + 1, 64], f32, tag="g2")
                          for (toff, ntl) in _chunks(K + 1):
                              nc.gpsimd.dma_gather(
                                  g2[:, toff : toff + ntl, :],
                                  h2full[BIAS:, :],
                                  idx2_sb[:, o8 + toff * 8 : o8 + (toff + ntl) * 8],
                                  ntl * P,
                                  ntl * P,
                                  64,
                                  queue_num=w % nqueues,
                              )
                          ev = g2[:, 1:, 2:3]
                          nc.vector.tensor_tensor(
                              out=ev, in0=ev,
                              in1=g2[:, 0, 3:4].unsqueeze(1).broadcast_to([P, K, 1]),
                              op=LR.add,
                          )
                          nc.vector.scalar_tensor_tensor(
                              out=ev, in0=ev, scalar=NEG, in1=ev,
                              op0=LR.mult, op1=LR.max,
                          )
                          rhs2 = pcs.tile([P, K, 3], f32, tag="rhs2")
                          nc.scalar.activation(rhs2[:, :, 2:3], ev, AF.Exp)
                          nc.vector.tensor_tensor(
                              out=rhs2[:, :, 0:2], in0=g2[:, 1:, 0:2],
                              in1=rhs2[:, :, 2:3].broadcast_to([P, K, 2]), op=LR.mult,
                          )
                          # segment sum over rounds = strided free-dim reduce (no PE)
                          agg2 = pcs.tile([P, 3], f32, tag="agg2")
                          nc.vector.reduce_sum(
                              agg2[:], rhs2[:].transpose([0, 2, 1]),
                              axis=mybir.AxisListType.X,
                          )
                          den = pcs.tile([P, 1], f32, tag="den2")
                          nc.vector.tensor_scalar(
                              out=den[:], in0=agg2[:, 2:3], scalar1=EPS,
                              scalar2=None, op0=LR.add,
                          )
                          rec = pcs.tile([P, 1], f32, tag="rec2")
                          nc.vector.reciprocal(rec[:], den[:])
                          o2 = pcs.tile([P, OUT], f32, tag="o2")
                          nc.vector.tensor_scalar(
                              out=o2[:], in0=agg2[:, 0:2], scalar1=rec[:, 0:1],
                              scalar2=None, op0=LR.mult,
                          )
                          nc.vector.tensor_tensor(out=o2[:], in0=o2[:], in1=b2_rep[:], op=LR.add)
                          nc.sync.dma_start(out=out_d[w * P : (w + 1) * P, :], in_=o2[:])

    nc.compile()
    return nc


def _preprocess(x, edge_index, W1, a_src1, a_dst1, b1, W2, a_src2, a_dst2, b2):
    src = np.concatenate([np.asarray(edge_index[0]), np.arange(N)]).astype(np.int64)
    dst = np.concatenate([np.asarray(edge_index[1]), np.arange(N)]).astype(np.int64)
    E2 = len(dst)

    deg = np.bincount(dst, minlength=N)
    # per-core degree-desc permutation, laid into a [W, 128] grid whose last
    # column (slot 127) is always a dummy
    perm = np.full((CORES, NPCP), -1, np.int64)
    posg = np.empty(N, np.int64)
    grid_real = (np.arange(NPCP) % P) < SL
    real_pos = np.nonzero(grid_real)[0]  # grid positions for real nodes, row-major
    for c in range(CORES):
        nodes = np.arange(c * NPC, (c + 1) * NPC)
        order = np.argsort(-deg[nodes], kind="stable")
        pc = nodes[order]
        perm[c, real_pos[:NPC]] = pc
        posg[pc] = real_pos[:NPC]

    # per-window rounds = max degree over the 8 cores' windows
    degw = np.zeros((CORES, W), np.int64)
    for c in range(CORES):
        dpad = np.zeros(NPCP, np.int64)
        m = perm[c] >= 0
        dpad[m] = deg[perm[c][m]]
        degw[c] = dpad.reshape(W, P).max(1)
    R = np.maximum(degw.max(0), 1)
    # per window: 1 own-row round + R edge rounds
    off = np.zeros(W + 1, np.int64)
    off[1:] = np.cumsum(R + 1)
    S = int(off[-1])

    # edge placement: round 1+r, slot s of window w on core c
    eorder = np.argsort(dst, kind="stable")
    starts = np.zeros(N, np.int64)
    starts[1:] = np.cumsum(deg)[:-1]
    d_sorted = dst[eorder]
    s_sorted = src[eorder]
    rank = np.arange(E2) - starts[d_sorted]
    c_e = d_sorted // NPC
    pos_e = posg[d_sorted]
    w_e = pos_e >> 7
    sl_e = pos_e & 127
    flat = (off[w_e] + 1 + rank) * P + sl_e

    idx1 = np.full((CORES, S * P), PADROW1, np.int64)
    idx1[c_e, flat] = s_sorted
    idx2 = np.full((CORES, S * P), PAD2, np.int64)
    idx2[c_e, flat] = (s_sorted // NPC) * NPCP2 + posg[s_sorted]
    # round 0 of each window: own rows (al_dst for layer 1, h2/al2_dst for layer 2)
    r0 = (off[:-1][:, None] * P + np.arange(P)[None, :]).reshape(-1)
    idx1[:, r0] = np.where(perm >= 0, perm, PADROW1)
    own_rows = np.arange(CORES)[:, None] * NPCP2 + np.arange(NPCP)[None, :]
    idx2[:, r0] = own_rows

    idx1 = _wrap_idx_stream(idx1)
    idx2 = _wrap_idx_stream(idx2)

    # weights
    W1 = np.asarray(W1, np.float32)
    W1r = W1.reshape(INCH, HEADS, HID)
    wa_s = np.einsum("ihc,hc->ih", W1r, np.asarray(a_src1, np.float32))
    wa_d = np.einsum("ihc,hc->ih", W1r, np.asarray(a_dst1, np.float32))
    wcat = np.concatenate([W1, wa_s, wa_d], axis=1).astype(ml_dtypes.bfloat16)

    W2 = np.asarray(W2, np.float32)
    w2s = W2 @ np.asarray(a_src2, np.float32)[0]
    w2d = W2 @ np.asarray(a_dst2, np.float32)[0]
    w2cat = np.concatenate([W2, w2s[:, None], w2d[:, None]], axis=1).astype(np.float32)
    w2cat = np.concatenate([w2cat[:P], w2cat[P:]], axis=1)  # [128, 8]

    x_pad = np.zeros((NROW1, INCH), np.float32)
    x_pad[:N] = np.asarray(x, np.float32)
    xT = np.ascontiguousarray(x_pad.T).astype(ml_dtypes.bfloat16)

    in_maps = []
    for c in range(CORES):
        in_maps.append(
            {
                "xT": xT,
                "wcat": wcat,
                "w2cat": w2cat,
                "b1": np.asarray(b1, np.float32).reshape(1, 256),
                "b2": np.asarray(b2, np.float32).reshape(1, 2),
                "idx1": idx1[c],
                "idx2": idx2[c],
            }
        )
    return tuple(int(r) for r in R), perm, in_maps


class _Runner:
    """Persistent compiled runner: jit once, device-resident inputs, so
    repeated calls time only execution (+ dispatch)."""

    def __init__(self, nc):
        import jax
        from jax.sharding import Mesh, PartitionSpec, NamedSharding
        from jax.experimental.shard_map import shard_map
        from concourse import bass2jax
        import concourse.mybir as mb

        bass2jax.install_neuronx_cc_hook()
        self.jax = jax
        self.nc = nc
        part_name = nc.partition_id_tensor.name if nc.partition_id_tensor else None
        in_names, out_names, out_avals, zero_outs = [], [], [], []
        for alloc in nc.m.functions[0].allocations:
            if not isinstance(alloc, mb.MemoryLocationSet):
                continue
            name = alloc.memorylocations[0].name
            if alloc.kind == "ExternalInput":
                if name != part_name:
                    in_names.append(name)
            elif alloc.kind == "ExternalOutput":
                out_names.append(name)
                shape = tuple(alloc.tensor_shape)
                dtype = mb.dt.np(alloc.dtype)
                out_avals.append(jax.core.ShapedArray(shape, dtype))
                zero_outs.append(np.zeros(shape, dtype))
        self.in_names, self.out_names = in_names, out_names
        self.zero_outs = zero_outs
        n_params, n_outs = len(in_names), len(out_names)
        donate = tuple(range(n_params, n_params + n_outs))

        all_in_names = in_names + out_names + ([part_name] if part_name else [])

        def _body(*args):
            operands = list(args)
            if part_name is not None:
                operands.append(bass2jax.partition_id_tensor())
            outs = bass2jax._bass_exec_p.bind(
                *operands,
                out_avals=tuple(out_avals),
                in_names=tuple(all_in_names),
                out_names=tuple(out_names),
                lowering_input_output_aliases=(),
                sim_require_finite=True,
                sim_require_nnan=True,
                nc=nc,
            )
            return tuple(outs)

        devices = jax.devices()[:CORES]
        self.mesh = Mesh(np.asarray(devices), ("core",))
        self.spec = NamedSharding(self.mesh, PartitionSpec("core"))
        in_specs = (PartitionSpec("core"),) * (n_params + n_outs)
        out_specs = (PartitionSpec("core"),) * n_outs
        self.sharded = jax.jit(
            shard_map(_body, mesh=self.mesh, in_specs=in_specs,
                      out_specs=out_specs, check_rep=False),
            donate_argnums=donate, keep_unused=True,
        )
        self.dev_in = None

    def put_inputs(self, in_maps):
        self.dev_in = [
            self.jax.device_put(
                np.concatenate([np.asarray(m[n]) for m in in_maps], axis=0), self.spec
            )
            for n in self.in_names
        ]
        self.jax.block_until_ready(self.dev_in)

    def execute(self):
        zeros = [
            self.jax.device_put(
                np.zeros((CORES * z.shape[0], *z.shape[1:]), z.dtype), self.spec
            )
            for z in self.zero_outs
        ]
        self.jax.block_until_ready(zeros)
        t0 = time.monotonic_ns()
        outs = self.sharded(*self.dev_in, *zeros)
        self.jax.block_until_ready(outs)
        dt = time.monotonic_ns() - t0
        res = [
            {
                name: np.asarray(outs[i]).reshape(CORES, *self.zero_outs[i].shape)[c]
                for i, name in enumerate(self.out_names)
            }
            for c in range(CORES)
        ]
        return res, dt


def run_on_device(in_maps, R):
    if R not in _cache:
        _cache[R] = _Runner(_build(R))
    runner = _cache[R]
    runner.put_inputs(in_maps)
    res, dt = runner.execute()
    global LAST_EXEC_NS
    LAST_EXEC_NS = dt
    return res


def kernel(x, edge_index, W1, a_src1, a_dst1, b1, W2, a_src2, a_dst2, b2):
    R, perm, in_maps = _preprocess(
        x, edge_index, W1, a_src1, a_dst1, b1, W2, a_src2, a_dst2, b2
    )
    res = run_on_device(in_maps, R)
    out = np.empty((N, OUT), np.float32)
    for c in range(CORES):
        m = perm[c] >= 0
        out[perm[c][m]] = res[c]["out"][m]
    return out



# revision 7
# speedup vs baseline: 43.3323x; 43.3323x over previous
"""Two-layer GAT (GATConv 128->64x4 concat, relu, GATConv 256->2) on 8 TRN2
NeuronCores, self-contained.

Sharding: edges partitioned by destination node; core c owns dst nodes
[c*6250, (c+1)*6250). Within a core, own nodes are sorted by in-degree and
grouped into 49 windows of 128 ("slots"). Edges of a node are laid out in
"rounds": round r, slot s holds the r-th in-edge of window-node s, so each
[128-edge] round tile is already dst-aligned — segment aggregation is a plain
PSUM accumulation with an identity stationary operand (no one-hot matmuls),
and al_dst comes from one 128-row gather per window instead of a per-edge
gather. Degree sorting makes rounds-per-window track the window's max degree,
so padding stays low. Padding edges point at a dedicated table row whose
al_src is -1e30, which drives exp(leakyrelu(e)) to zero.

Device pipeline per core:
  Phase A: tab[g] = [x@W1 as bf16 (512B) | al_src f32[4] | al_dst f32[4] | pad]
           (768B rows) for ALL nodes, from a host-pretransposed bf16 x.
  Phase B: per window: gather src rows, e = al_s[src] + al_d[dst](broadcast),
           p = exp(leaky_relu(e)), msg = h[src]*p (bf16), PSUM += msg via
           identity matmul; readout: alpha-normalize, +b1, relu,
           h2lite = relu1 @ [W2|W2 a_s2|W2 a_d2] -> h2own rows.
  AllGather h2own -> h2full. Phase C: same structure for layer 2.
"""

import os
import sys
import time

sys.path.insert(0, "/opt/trn_rl_repo")

import numpy as np
import ml_dtypes

import concourse.bacc as bacc
import concourse.mybir as mybir
import concourse.tile as tile
from concourse.masks import make_identity

# problem constants (hardcoded per harness contract)
N = 50000
INCH = 128
HID = 64
HEADS = 4
OUT = 2
NEG = 0.2
CORES = 8
NPC = N // CORES          # 6250 dst nodes per core
P = 128
SL = 127                  # real slots per window; slot 127 is always padding so
                          # every gather chunk's final int16 index is >= 0
W = 50                    # windows per core (50*127 = 6350 >= 6250)
NPCP = W * P              # padded nodes per core (6400)
NPCP2 = NPCP + P          # + one pad-row block (6528)
NROW1 = 393 * P           # l1 table rows (50304 > N)
NROW2 = CORES * NPCP2     # h2 table rows (52224)
PADROW1 = N               # tab row used by padding edges (al_s = -1e30); >= 32768
PAD2 = 5 * NPCP2 + NPCP   # h2full pad row inside core 5's block; >= 32768
BIAS = 32768              # int16 gather index bias
EPS = 1e-16
ROWF = 192                # tab row width in f32 (768 B)

f32 = mybir.dt.float32
bf16 = mybir.dt.bfloat16
i16 = mybir.dt.int16

LAST_EXEC_NS = None
_cache = {}


def _wrap_idx_stream(a):
    """[C, M] int64 -> biased int16 [C, 128, M//16] (16-partition wrap,
    replicated to all 8 Q7 groups)."""
    Cn, M = a.shape
    b = (a - BIAS).astype(np.int16).reshape(Cn, M // 16, 16).transpose(0, 2, 1)
    return np.tile(b, (1, 8, 1)).copy()


CHUNK = int(os.environ.get("KCHUNK", "8"))


def _chunks(K):
    """[(tile_off, ntiles)] with ntiles <= CHUNK (dma_gather idx limit)."""
    out = []
    off = 0
    while off < K:
        n = min(CHUNK, K - off)
        out.append((off, n))
        off += n
    return out


def _build(R):
    R = list(R)
    # per window: 1 own-row round (al_dst / h2own values) + R edge rounds
    off = np.zeros(W + 1, np.int64)
    off[1:] = np.cumsum(np.asarray(R) + 1)
    S = int(off[-1])
    phases = os.environ.get("KPHASES", "ABGC")
    nqueues = int(os.environ.get("KQUEUES", "2"))
    nc = bacc.Bacc(
        "TRN2", target_bir_lowering=False, debug=False, num_devices=CORES,
        num_swdge_queues=nqueues,
    )

    # inputs
    xT_d = nc.dram_tensor("xT", [INCH, NROW1], bf16, kind="ExternalInput")
    wcat_d = nc.dram_tensor("wcat", [INCH, 264], bf16, kind="ExternalInput")
    w2cat_d = nc.dram_tensor("w2cat", [P, 8], f32, kind="ExternalInput")
    b1_d = nc.dram_tensor("b1", [1, 256], f32, kind="ExternalInput")
    b2_d = nc.dram_tensor("b2", [1, 2], f32, kind="ExternalInput")
    idx1_d = nc.dram_tensor("idx1", [P, S * 8], i16, kind="ExternalInput")
    idx2_d = nc.dram_tensor("idx2", [P, S * 8], i16, kind="ExternalInput")

    out_d = nc.dram_tensor("out", [NPCP, OUT], f32, kind="ExternalOutput")

    # scratch
    tab = nc.dram_tensor("tab", [NROW1, ROWF], f32)
    h2own = nc.dram_tensor("h2own", [NPCP2, 64], f32)
    h2full = nc.dram_tensor("h2full", [NROW2, 64], f32, addr_space="Shared")

    LR = mybir.AluOpType
    AF = mybir.ActivationFunctionType

    with tile.TileContext(nc) as tc:
        with tc.tile_pool(name="const", bufs=1) as cpool:
            ident_b = cpool.tile([P, P], bf16)
            make_identity(nc, ident_b[:])
            ident_f = cpool.tile([P, P], f32)
            make_identity(nc, ident_f[:])
            ones = cpool.tile([1, P], f32)
            nc.vector.memset(ones[:], 1.0)

            wcat_sb = cpool.tile([INCH, 264], bf16)
            nc.sync.dma_start(out=wcat_sb[:], in_=wcat_d[:, :])
            w2cat_sb = cpool.tile([P, 8], f32)
            nc.sync.dma_start(out=w2cat_sb[:], in_=w2cat_d[:, :])
            b1row = cpool.tile([1, 256], f32)
            nc.sync.dma_start(out=b1row[:], in_=b1_d[:, :])
            b2row = cpool.tile([1, 2], f32)
            nc.sync.dma_start(out=b2row[:], in_=b2_d[:, :])
            idx1_sb = cpool.tile([P, S * 8], i16)
            nc.sync.dma_start(out=idx1_sb[:], in_=idx1_d[:, :])
            idx2_sb = cpool.tile([P, S * 8], i16)
            nc.sync.dma_start(out=idx2_sb[:], in_=idx2_d[:, :])

            # pad-row payload: al_s = al_d = -1e30, h = 0
            padrow = cpool.tile([1, ROWF], f32)
            nc.vector.memset(padrow[:], 0.0)
            nc.vector.memset(padrow[:, 128:136], -1e30)
            pad2 = cpool.tile([P, 4], f32)
            nc.vector.memset(pad2[:], 0.0)
            nc.vector.memset(pad2[:, 2:4], -1e30)

            # replicated biases
            with tc.tile_pool(name="psum_b", bufs=1, space="PSUM") as psb:
                b1_ps = psb.tile([P, 256], f32, space="PSUM")
                nc.tensor.matmul(out=b1_ps[:], lhsT=ones[:], rhs=b1row[:], start=True, stop=True)
                b1_rep = cpool.tile([P, 256], f32)
                nc.vector.tensor_copy(b1_rep[:], b1_ps[:])
                b2_ps = psb.tile([P, 2], f32, space="PSUM")
                nc.tensor.matmul(out=b2_ps[:], lhsT=ones[:], rhs=b2row[:], start=True, stop=True)
                b2_rep = cpool.tile([P, 2], f32)
                nc.vector.tensor_copy(b2_rep[:], b2_ps[:])

            reps = int(os.environ.get("KREPS", "1"))
            for _rep in range(reps):
              # ---------------- Phase A: node features table ----------------
              if "A" in phases:
                  with (
                      tc.tile_pool(name="sbufA", bufs=4) as pa,
                      tc.tile_pool(name="psumA", bufs=4, space="PSUM") as ppa,
                  ):
                      ntile = NROW1 // P
                      GA = 6
                      for i0 in range(0, ntile, GA):
                          g = min(GA, ntile - i0)
                          xt = pa.tile([INCH, GA * P], bf16, tag="xt")
                          nc.sync.dma_start(
                              out=xt[:, 0 : g * P],
                              in_=xT_d[:, i0 * P : (i0 + g) * P],
                          )
                          stg = pa.tile([P, GA, ROWF], f32, tag="stg")
                          for j in range(g):
                              h_ps = ppa.tile([P, 264], f32, space="PSUM")
                              nc.tensor.matmul(
                                  out=h_ps[:], lhsT=xt[:, j * P : (j + 1) * P],
                                  rhs=wcat_sb[:], start=True, stop=True,
                              )
                              nc.vector.tensor_copy(
                                  stg[:, j, 0:128].bitcast(bf16), h_ps[:, 0:256]
                              )
                              nc.vector.tensor_copy(stg[:, j, 128:136], h_ps[:, 256:264])
                          # cols 136:192 are padding: written as-is, never read back
                          nc.sync.dma_start(
                              out=tab[i0 * P : (i0 + g) * P, :].rearrange(
                                  "(g p) c -> p g c", g=g
                              ),
                              in_=stg[:, 0:g],
                          )
                      # overwrite the pad row (al_s/al_d = -1e30)
                      nc.sync.dma_start(out=tab[PADROW1 : PADROW1 + 1, :], in_=padrow[:])

              # ---------------- Phase B: layer-1 edge aggregation ----------------
              if "B" in phases:
                  with (
                      tc.tile_pool(name="sbufB", bufs=3) as pb,
                      tc.tile_pool(name="sbufBp", bufs=2) as pbp,
                      tc.tile_pool(name="sbufBs", bufs=4) as pbs,
                      tc.tile_pool(name="psumAgg", bufs=3, space="PSUM") as pagg,
                      tc.tile_pool(name="psumT", bufs=2, space="PSUM") as pt,
                      tc.tile_pool(name="psumH", bufs=2, space="PSUM") as ph,
                  ):
                      # pad block of h2own (rows NPCP..NPCP2)
                      nc.sync.dma_start(out=h2own[NPCP:NPCP2, 0:4], in_=pad2[:])
                      h2all = pbp.tile([P, W, 4], f32, tag="h2all")
                      for w in range(W):
                          K = R[w]
                          o8 = int(off[w]) * 8
                          gbuf = pb.tile([P, K + 1, ROWF], f32, tag="gbuf")
                          for (toff, ntl) in _chunks(K + 1):
                              nc.gpsimd.dma_gather(
                                  gbuf[:, toff : toff + ntl, :],
                                  tab[BIAS:, :],
                                  idx1_sb[:, o8 + toff * 8 : o8 + (toff + ntl) * 8],
                                  ntl * P,
                                  ntl * P,
                                  ROWF,
                                  queue_num=w % nqueues,
                              )
                          # e = al_s[src] + al_d[dst] (round 0 = own rows), lrelu, exp
                          alv = gbuf[:, 1:, 128:132]
                          nc.vector.tensor_tensor(
                              out=alv, in0=alv,
                              in1=gbuf[:, 0, 132:136].unsqueeze(1).broadcast_to([P, K, 4]),
                              op=LR.add,
                          )
                          nc.vector.scalar_tensor_tensor(
                              out=alv, in0=alv, scalar=NEG, in1=alv,
                              op0=LR.mult, op1=LR.max,
                          )
                          p32 = pbs.tile([P, K, 4], f32, tag="p32")
                          nc.scalar.activation(p32[:], alv, AF.Exp)
                          pcat = pbp.tile([P, K, HEADS, HID], bf16, tag="pcat")
                          hview4 = gbuf[:, 1:, 0:128].bitcast(bf16).rearrange(
                              "p k (h c) -> p k h c", h=HEADS
                          )
                          for k in range(K):
                              nc.vector.tensor_tensor(
                                  out=pcat[:, k],
                                  in0=hview4[:, k],
                                  in1=p32[:, k, :].unsqueeze(2).broadcast_to(
                                      [P, HEADS, HID]
                                  ),
                                  op=LR.mult,
                              )
                          # denominators: den4[p,h] = sum_k p32[p,k,h]
                          den4 = pbs.tile([P, 4], f32, tag="den4")
                          nc.vector.reduce_sum(
                              den4[:], p32[:].transpose([0, 2, 1]),
                              axis=mybir.AxisListType.X,
                          )
                          agg_ps = pagg.tile([P, 256], f32, space="PSUM", tag="agg")
                          for k in range(K):
                              nc.tensor.matmul(
                                  out=agg_ps[:], lhsT=ident_b[:],
                                  rhs=pcat[:, k].opt(),
                                  start=(k == 0), stop=(k == K - 1),
                              )
                          # window readout
                          den = pbs.tile([P, 4], f32, tag="den")
                          nc.vector.tensor_scalar(
                              out=den[:], in0=den4[:], scalar1=EPS,
                              scalar2=None, op0=LR.add,
                          )
                          rec = pbs.tile([P, 4], f32, tag="rec")
                          nc.vector.reciprocal(rec[:], den[:])
                          relu1 = pbs.tile([P, 256], f32, tag="relu1")
                          for h in range(HEADS):
                              if h < 2:
                                  nc.vector.tensor_scalar(
                                      out=relu1[:, h * HID : (h + 1) * HID],
                                      in0=agg_ps[:, h * HID : (h + 1) * HID],
                                      scalar1=rec[:, h : h + 1],
                                      scalar2=None,
                                      op0=LR.mult,
                                  )
                              else:
                                  nc.scalar.mul(
                                      relu1[:, h * HID : (h + 1) * HID],
                                      agg_ps[:, h * HID : (h + 1) * HID],
                                      rec[:, h : h + 1],
                                  )
                          nc.vector.tensor_tensor(
                              out=relu1[:], in0=relu1[:], in1=b1_rep[:], op=LR.add
                          )
                          nc.scalar.activation(relu1[:], relu1[:], AF.Relu)
                          h2_ps = ph.tile([P, 4], f32, space="PSUM", tag="h2")
                          for half in range(2):
                              rT_ps = pt.tile([P, P], f32, space="PSUM", tag="ohT")
                              nc.tensor.transpose(
                                  out=rT_ps[:], in_=relu1[:, half * P : (half + 1) * P],
                                  identity=ident_f[:],
                              )
                              rT = pbs.tile([P, P], f32, tag="ohTs")
                              nc.vector.tensor_copy(rT[:], rT_ps[:])
                              nc.tensor.matmul(
                                  out=h2_ps[:], lhsT=rT[:],
                                  rhs=w2cat_sb[:, half * 4 : (half + 1) * 4],
                                  start=(half == 0), stop=(half == 1),
                              )
                          nc.vector.tensor_copy(h2all[:, w], h2_ps[:])
                      nc.sync.dma_start(
                          out=h2own[0:NPCP, 0:4].rearrange("(w p) c -> p w c", w=W),
                          in_=h2all[:],
                      )

              # ---------------- AllGather h2lite ----------------
              if "G" in phases:
                  nc.gpsimd.collective_compute(
                      "AllGather",
                      mybir.AluOpType.bypass,
                      replica_groups=[list(range(CORES))],
                      ins=[h2own.ap().opt()],
                      outs=[h2full.ap().opt()],
                  )

              # ---------------- Phase C: layer-2 edge aggregation ----------------
              if "C" in phases:
                  with (
                      tc.tile_pool(name="sbufC", bufs=3) as pc,
                      tc.tile_pool(name="sbufCs", bufs=4) as pcs,
                  ):
                      o2all = pc.tile([P, W, OUT], f32, tag="o2all")
                      for w in range(W):
                          K = R[w]
                          o8 = int(off[w]) * 8
                          g2 = pc.tile([P, K + 1, 64], f32, tag="g2")
                          for (toff, ntl) in _chunks(K + 1):
                              nc.gpsimd.dma_gather(
                                  g2[:, toff : toff + ntl, :],
                                  h2full[BIAS:, :],
                                  idx2_sb[:, o8 + toff * 8 : o8 + (toff + ntl) * 8],
                                  ntl * P,
                                  ntl * P,
                                  64,
                                  queue_num=w % nqueues,
                              )
                          ev = g2[:, 1:, 2:3]
                          nc.vector.tensor_tensor(
                              out=ev, in0=ev,
                              in1=g2[:, 0, 3:4].unsqueeze(1).broadcast_to([P, K, 1]),
                              op=LR.add,
                          )
                          nc.vector.scalar_tensor_tensor(
                              out=ev, in0=ev, scalar=NEG, in1=ev,
                              op0=LR.mult, op1=LR.max,
                          )
                          rhs2 = pcs.tile([P, K, 3], f32, tag="rhs2")
                          nc.scalar.activation(rhs2[:, :, 2:3], ev, AF.Exp)
                          nc.vector.tensor_tensor(
                              out=rhs2[:, :, 0:2], in0=g2[:, 1:, 0:2],
                              in1=rhs2[:, :, 2:3].broadcast_to([P, K, 2]), op=LR.mult,
                          )
                          # segment sum over rounds = strided free-dim reduce (no PE)
                          agg2 = pcs.tile([P, 3], f32, tag="agg2")
                          nc.vector.reduce_sum(
                              agg2[:], rhs2[:].transpose([0, 2, 1]),
                              axis=mybir.AxisListType.X,
                          )
                          den = pcs.tile([P, 1], f32, tag="den2")
                          nc.vector.tensor_scalar(
                              out=den[:], in0=agg2[:, 2:3], scalar1=EPS,
                              scalar2=None, op0=LR.add,
                          )
                          rec = pcs.tile([P, 1], f32, tag="rec2")
                          nc.vector.reciprocal(rec[:], den[:])
                          nc.vector.tensor_scalar(
                              out=o2all[:, w], in0=agg2[:, 0:2], scalar1=rec[:, 0:1],
                              scalar2=None, op0=LR.mult,
                          )
                          nc.vector.tensor_tensor(
                              out=o2all[:, w], in0=o2all[:, w], in1=b2_rep[:], op=LR.add
                          )
                      nc.sync.dma_start(
                          out=out_d[:, :].rearrange("(w p) c -> p w c", w=W),
                          in_=o2all[:],
                      )

    nc.compile()
    return nc


def _preprocess(x, edge_index, W1, a_src1, a_dst1, b1, W2, a_src2, a_dst2, b2):
    src = np.concatenate([np.asarray(edge_index[0]), np.arange(N)]).astype(np.int64)
    dst = np.concatenate([np.asarray(edge_index[1]), np.arange(N)]).astype(np.int64)
    E2 = len(dst)

    deg = np.bincount(dst, minlength=N)
    # per-core degree-desc permutation, laid into a [W, 128] grid whose last
    # column (slot 127) is always a dummy
    perm = np.full((CORES, NPCP), -1, np.int64)
    posg = np.empty(N, np.int64)
    grid_real = (np.arange(NPCP) % P) < SL
    real_pos = np.nonzero(grid_real)[0]  # grid positions for real nodes, row-major
    for c in range(CORES):
        nodes = np.arange(c * NPC, (c + 1) * NPC)
        order = np.argsort(-deg[nodes], kind="stable")
        pc = nodes[order]
        perm[c, real_pos[:NPC]] = pc
        posg[pc] = real_pos[:NPC]

    # per-window rounds = max degree over the 8 cores' windows
    degw = np.zeros((CORES, W), np.int64)
    for c in range(CORES):
        dpad = np.zeros(NPCP, np.int64)
        m = perm[c] >= 0
        dpad[m] = deg[perm[c][m]]
        degw[c] = dpad.reshape(W, P).max(1)
    R = np.maximum(degw.max(0), 1)
    # per window: 1 own-row round + R edge rounds
    off = np.zeros(W + 1, np.int64)
    off[1:] = np.cumsum(R + 1)
    S = int(off[-1])

    # edge placement: round 1+r, slot s of window w on core c
    eorder = np.argsort(dst, kind="stable")
    starts = np.zeros(N, np.int64)
    starts[1:] = np.cumsum(deg)[:-1]
    d_sorted = dst[eorder]
    s_sorted = src[eorder]
    rank = np.arange(E2) - starts[d_sorted]
    c_e = d_sorted // NPC
    pos_e = posg[d_sorted]
    w_e = pos_e >> 7
    sl_e = pos_e & 127
    flat = (off[w_e] + 1 + rank) * P + sl_e

    idx1 = np.full((CORES, S * P), PADROW1, np.int64)
    idx1[c_e, flat] = s_sorted
    idx2 = np.full((CORES, S * P), PAD2, np.int64)
    idx2[c_e, flat] = (s_sorted // NPC) * NPCP2 + posg[s_sorted]
    # round 0 of each window: own rows (al_dst for layer 1, h2/al2_dst for layer 2)
    r0 = (off[:-1][:, None] * P + np.arange(P)[None, :]).reshape(-1)
    idx1[:, r0] = np.where(perm >= 0, perm, PADROW1)
    own_rows = np.arange(CORES)[:, None] * NPCP2 + np.arange(NPCP)[None, :]
    idx2[:, r0] = own_rows

    idx1 = _wrap_idx_stream(idx1)
    idx2 = _wrap_idx_stream(idx2)

    # weights
    W1 = np.asarray(W1, np.float32)
    W1r = W1.reshape(INCH, HEADS, HID)
    wa_s = np.einsum("ihc,hc->ih", W1r, np.asarray(a_src1, np.float32))
    wa_d = np.einsum("ihc,hc->ih", W1r, np.asarray(a_dst1, np.float32))
    wcat = np.concatenate([W1, wa_s, wa_d], axis=1).astype(ml_dtypes.bfloat16)

    W2 = np.asarray(W2, np.float32)
    w2s = W2 @ np.asarray(a_src2, np.float32)[0]
    w2d = W2 @ np.asarray(a_dst2, np.float32)[0]
    w2cat = np.concatenate([W2, w2s[:, None], w2d[:, None]], axis=1).astype(np.float32)
    w2cat = np.concatenate([w2cat[:P], w2cat[P:]], axis=1)  # [128, 8]

    x_pad = np.zeros((NROW1, INCH), np.float32)
    x_pad[:N] = np.asarray(x, np.float32)
    xT = np.ascontiguousarray(x_pad.T).astype(ml_dtypes.bfloat16)

    in_maps = []
    for c in range(CORES):
        in_maps.append(
            {
                "xT": xT,
                "wcat": wcat,
                "w2cat": w2cat,
                "b1": np.asarray(b1, np.float32).reshape(1, 256),
                "b2": np.asarray(b2, np.float32).reshape(1, 2),
                "idx1": idx1[c],
                "idx2": idx2[c],
            }
        )
    return tuple(int(r) for r in R), perm, in_maps


class _Runner:
    """Persistent compiled runner: jit once, device-resident inputs, so
    repeated calls time only execution (+ dispatch)."""

    def __init__(self, nc):
        import jax
        from jax.sharding import Mesh, PartitionSpec, NamedSharding
        from jax.experimental.shard_map import shard_map
        from concourse import bass2jax
        import concourse.mybir as mb

        bass2jax.install_neuronx_cc_hook()
        self.jax = jax
        self.nc = nc
        part_name = nc.partition_id_tensor.name if nc.partition_id_tensor else None
        in_names, out_names, out_avals, zero_outs = [], [], [], []
        for alloc in nc.m.functions[0].allocations:
            if not isinstance(alloc, mb.MemoryLocationSet):
                continue
            name = alloc.memorylocations[0].name
            if alloc.kind == "ExternalInput":
                if name != part_name:
                    in_names.append(name)
            elif alloc.kind == "ExternalOutput":
                out_names.append(name)
                shape = tuple(alloc.tensor_shape)
                dtype = mb.dt.np(alloc.dtype)
                out_avals.append(jax.core.ShapedArray(shape, dtype))
                zero_outs.append(np.zeros(shape, dtype))
        self.in_names, self.out_names = in_names, out_names
        self.zero_outs = zero_outs
        n_params, n_outs = len(in_names), len(out_names)
        donate = tuple(range(n_params, n_params + n_outs))

        all_in_names = in_names + out_names + ([part_name] if part_name else [])

        def _body(*args):
            operands = list(args)
            if part_name is not None:
                operands.append(bass2jax.partition_id_tensor())
            outs = bass2jax._bass_exec_p.bind(
                *operands,
                out_avals=tuple(out_avals),
                in_names=tuple(all_in_names),
                out_names=tuple(out_names),
                lowering_input_output_aliases=(),
                sim_require_finite=True,
                sim_require_nnan=True,
                nc=nc,
            )
            return tuple(outs)

        devices = jax.devices()[:CORES]
        self.mesh = Mesh(np.asarray(devices), ("core",))
        self.spec = NamedSharding(self.mesh, PartitionSpec("core"))
        in_specs = (PartitionSpec("core"),) * (n_params + n_outs)
        out_specs = (PartitionSpec("core"),) * n_outs
        self.sharded = jax.jit(
            shard_map(_body, mesh=self.mesh, in_specs=in_specs,
                      out_specs=out_specs, check_rep=False),
            donate_argnums=donate, keep_unused=True,
        )
        self.dev_in = None

    def put_inputs(self, in_maps):
        self.dev_in = [
            self.jax.device_put(
                np.concatenate([np.asarray(m[n]) for m in in_maps], axis=0), self.spec
            )
            for n in self.in_names
        ]
        self.jax.block_until_ready(self.dev_in)

    def execute(self):
        zeros = [
            self.jax.device_put(
                np.zeros((CORES * z.shape[0], *z.shape[1:]), z.dtype), self.spec
            )
            for z in self.zero_outs
        ]
        self.jax.block_until_ready(zeros)
        t0 = time.monotonic_ns()
        outs = self.sharded(*self.dev_in, *zeros)
        self.jax.block_until_ready(outs)
        dt = time.monotonic_ns() - t0
        res = [
            {
                name: np.asarray(outs[i]).reshape(CORES, *self.zero_outs[i].shape)[c]
                for i, name in enumerate(self.out_names)
            }
            for c in range(CORES)
        ]
        return res, dt


def run_on_device(in_maps, R):
    if R not in _cache:
        _cache[R] = _Runner(_build(R))
    runner = _cache[R]
    runner.put_inputs(in_maps)
    res, dt = runner.execute()
    global LAST_EXEC_NS
    LAST_EXEC_NS = dt
    return res


def kernel(x, edge_index, W1, a_src1, a_dst1, b1, W2, a_src2, a_dst2, b2):
    R, perm, in_maps = _preprocess(
        x, edge_index, W1, a_src1, a_dst1, b1, W2, a_src2, a_dst2, b2
    )
    res = run_on_device(in_maps, R)
    out = np.empty((N, OUT), np.float32)
    for c in range(CORES):
        m = perm[c] >= 0
        out[perm[c][m]] = res[c]["out"][m]
    return out

